# revision 1
# baseline (speedup 1.0000x reference)
"""MultiHeadAttention Trainium2 kernel (8 NeuronCores).

Sharding: core c -> (batch b = c//2, head-group g = c%2) of the 12 heads.
Each core computes attention for its 6 heads of one batch element and a
partial projection; the host sums the two head-group partials per batch
element and adds proj bias.

Per-core dataflow (feat-major / transposed layouts to avoid transposes):
  x [S,768] --PE-transpose--> xT [768,S]
  qT/kT = (wqk^T @ xT) + bias     (float32r matmuls, TF32-class)
  v [S,384] seq-major (+ ones column for softmax denominators)
  scoresT[sk,sq] = kT_chunk^T-pair @ qT  (2 heads packed in PE row groups)
  pT = exp(scoresT/8)              (ScalarE, no max subtraction - bounded)
  avT[65,sq] = [v|1]^T @ pT        (row 64 = softmax denominator)
  attn_outT = avT[0:64] * (1/avT[64]) broadcast via K=1 PE matmul
  yT[768,S] = wp^T @ attn_outT     (partial projection, host sums pairs)
"""
import sys

sys.path.insert(0, "/opt/trn_rl_repo")

import numpy as np

import concourse.bass as bass
import concourse.mybir as mybir
import concourse.tile as tile
from concourse import bacc
from concourse.bass_utils import run_bass_kernel_spmd
from concourse.masks import make_identity

F32 = mybir.dt.float32
F32R = mybir.dt.float32r
EXP = mybir.ActivationFunctionType.Exp
ADD = mybir.AluOpType.add

HID = 768
D = 64  # head dim
LHEADS = 6  # heads per core
PAIRS = 3


def build_nc(S: int, taps: bool = False):
    nc = bacc.Bacc("TRN2", target_bir_lowering=False, debug=False)
    NSEQ = S // 128  # seq chunks of 128
    NBLK = S // 512  # seq blocks of 512
    XG = 4  # x DMA chunk groups
    NXG = NSEQ // XG

    x = nc.dram_tensor("x", [S, HID], F32, kind="ExternalInput")
    wqk = nc.dram_tensor("wqk", [HID, 768], F32, kind="ExternalInput")
    wv = nc.dram_tensor("wv", [HID, 384], F32, kind="ExternalInput")
    bqk = nc.dram_tensor("bqk", [768], F32, kind="ExternalInput")
    bv = nc.dram_tensor("bv", [384], F32, kind="ExternalInput")
    wp = nc.dram_tensor("wp", [384, HID], F32, kind="ExternalInput")
    yT = nc.dram_tensor("yT", [HID, S], F32, kind="ExternalOutput")
    if taps:
        t_xT = nc.dram_tensor("t_xT", [128, 6 * S], F32, kind="ExternalOutput")
        t_qT = nc.dram_tensor("t_qT", [128, S], F32, kind="ExternalOutput")
        t_kT = nc.dram_tensor("t_kT", [128, S], F32, kind="ExternalOutput")
        t_v = nc.dram_tensor("t_v", [128, S // 128 * 2 * 65], F32, kind="ExternalOutput")
        t_pt = nc.dram_tensor("t_pt", [128, 1024], F32, kind="ExternalOutput")
        t_av = nc.dram_tensor("t_av", [65, 512], F32, kind="ExternalOutput")
        t_ao = nc.dram_tensor("t_ao", [128, 3 * S], F32, kind="ExternalOutput")
        t_rc = nc.dram_tensor("t_rc", [1, 512], F32, kind="ExternalOutput")
        t_bc = nc.dram_tensor("t_bc", [D, 512], F32, kind="ExternalOutput")

    with tile.TileContext(nc) as tc:
        with (
            tc.tile_pool(name="const", bufs=1) as cp,
            tc.tile_pool(name="wts", bufs=1) as wpool,
            tc.tile_pool(name="ao", bufs=1) as aop,
            tc.tile_pool(name="ps", bufs=2, space="PSUM") as ps,
        ):
            ident = cp.tile([128, 128], F32, tag="ident")
            make_identity(nc, ident[:])
            ones_f = cp.tile([33, 128], F32, tag="onesf")
            nc.vector.memset(ones_f[:], 1.0)
            ones_r = cp.tile([33, 128], F32R, tag="ones")
            nc.vector.tensor_copy(ones_r[:], ones_f[:])
            bqk_sb = cp.tile([128, 6], F32, tag="bqk")
            nc.sync.dma_start(bqk_sb[:], bqk[:].rearrange("(c p) -> p c", p=128))
            bv_sb = cp.tile([1, 384], F32, tag="bvs")
            nc.sync.dma_start(bv_sb[:], bv[:].rearrange("(o f) -> o f", o=1))
            bv_r = cp.tile([1, 384], F32R, tag="bvr")
            nc.vector.tensor_copy(bv_r[:], bv_sb[:])
            # load the exp ACT table off the critical path
            warm = cp.tile([1, 16], F32, tag="warm")
            nc.scalar.activation(warm[:], ones_f[0:1, 0:16], EXP, bias=0.0, scale=0.0)

            aoT = aop.tile([128, PAIRS, S], F32R, tag="aoT")

            with tc.tile_pool(name="xT", bufs=1) as xtp:
                xT = xtp.tile([128, 6, S], F32R, tag="xT")

                # --- x DMA (group 0 first), weights, transposes, qkT(0), v interleaved ---
                with tc.tile_pool(name="wstage", bufs=1) as wst, \
                     tc.tile_pool(name="xin", bufs=1) as xin:
                    x_ap = x[:].rearrange("(n p) d -> p n d", p=128)
                    x_ts = []
                    for g in range(XG):
                        x_t = xin.tile([128, NXG, HID], F32, tag=f"x{g}", name=f"x_t{g}")
                        x_ts.append(x_t)
                    nc.sync.dma_start(x_ts[0][:], x_ap[:, 0:NXG, :])
                    wqk_f = wst.tile([128, 6, 768], F32, tag="wqkf")
                    wqk_ap = wqk[:].rearrange("(c p) f -> p c f", p=128)
                    for kc in range(6):
                        nc.sync.dma_start(
                            wqk_f[:, kc : kc + 1, :], wqk_ap[:, kc : kc + 1, :]
                        )
                    for g in range(1, XG):
                        nc.sync.dma_start(
                            x_ts[g][:], x_ap[:, g * NXG : (g + 1) * NXG, :]
                        )
                    wv_f = wst.tile([128, 6, 384], F32, tag="wvf")
                    nc.sync.dma_start(
                        wv_f[:], wv[:].rearrange("(c p) f -> p c f", p=128)
                    )

                    wqk_r = wpool.tile([128, 6, 768], F32R, tag="wqkr")
                    for kc in range(6):
                        nc.vector.tensor_copy(
                            wqk_r[:, kc, :], wqk_f[:, kc, :]
                        )
                    wv_r = wpool.tile([128, 6, 384], F32R, tag="wvr")
                    nc.vector.tensor_copy(wv_r[:], wv_f[:])


                    # transposes: per x-group, per hid-chunk j, 4 seq chunks into
                    # one psum tile, then a single [128, 512] contiguous copy
                    for g in range(XG):
                        for j in range(6):
                            tp = ps.tile([128, NXG, 128], F32, tag="av", bufs=2)
                            for i in range(NXG):
                                nc.tensor.transpose(
                                    tp[:, i, :],
                                    x_ts[g][:, i, j * 128 : (j + 1) * 128],
                                    ident[:],
                                )
                            dst = xT[:, j, g * NXG * 128 : (g + 1) * NXG * 128]
                            if (g * 6 + j) % 2 == 0:
                                nc.scalar.copy(dst, tp[:])
                            else:
                                nc.vector.tensor_copy(dst, tp[:])

                if taps:
                    nc.sync.dma_start(t_xT[:], xT[:].bitcast(F32).rearrange("p a b -> p (a b)"))
                den_init = [0]
                with (
                    tc.tile_pool(name="qk", bufs=2) as qkp,
                    tc.tile_pool(name="vv", bufs=2) as vvp,
                    tc.tile_pool(name="pt", bufs=4) as ptp,
                    tc.tile_pool(name="sm", bufs=2) as smp,
                ):
                    vsl = None
                    for pj in range(PAIRS):
                        # ---- qT/kT for this pair: [128 feats, S] f32r ----
                        qTp = qkp.tile([128, S], F32R, tag="qT")
                        kTp = qkp.tile([128, S], F32R, tag="kT")
                        for n in range(NBLK):
                            for dst, wcol in ((kTp, 3 + pj), (qTp, pj)):
                                qp = ps.tile([128, 512], F32, tag="qk", bufs=1)
                                for k in range(6):
                                    nc.tensor.matmul(
                                        qp[:],
                                        wqk_r[:, k, wcol * 128 : (wcol + 1) * 128],
                                        xT[:, k, n * 512 : (n + 1) * 512],
                                        start=(k == 0),
                                        stop=(k == 5),
                                    )
                                nc.vector.tensor_scalar(
                                    dst[:, n * 512 : (n + 1) * 512],
                                    qp[:],
                                    bqk_sb[:, wcol : wcol + 1],
                                    None,
                                    ADD,
                                )

                        if pj == 1:
                            wp_r = wpool.tile([128, 3, HID], F32R, tag="wpr")
                            wp_ap = wp[:].rearrange("(c p) f -> p c f", p=128)
                            for kc in range(3):
                                wp_f = smp.tile(
                                    [128, 1, HID], F32, tag="wpf", bufs=1,
                                    name=f"wp_f{kc}",
                                )
                                nc.sync.dma_start(wp_f[:], wp_ap[:, kc : kc + 1, :])
                                nc.vector.tensor_copy(wp_r[:, kc : kc + 1, :], wp_f[:])

                        if pj == 0:
                            # ---- v for all 6 heads (emitted after pair-0 qkT) ----
                            vsl = vvp.tile([128, NSEQ, 6, D + 1], F32R, tag="v", bufs=1)
                            vones = smp.tile([128, NSEQ, 6, 1], F32, tag="vones")
                            nc.vector.memset(vones[:], 1.0)
                            nc.vector.tensor_copy(vsl[:, :, :, D : D + 1], vones[:])
                            for i in range(NSEQ):
                                vp = ps.tile([128, 512], F32, tag="qk", bufs=1)
                                for k in range(6):
                                    nc.tensor.matmul(
                                        vp[:, 0:384],
                                        xT[:, k, i * 128 : (i + 1) * 128],
                                        wv_r[:, k, :],
                                        start=(k == 0),
                                        stop=False,
                                    )
                                nc.tensor.matmul(
                                    vp[:, 0:384],
                                    ones_r[0:1, :],
                                    bv_r[0:1, :],
                                    start=False,
                                    stop=True,
                                )
                                nc.vector.tensor_copy(
                                    vsl[:, i, :, 0:D],
                                    vp[:, 0:384].rearrange("p (h d) -> p h d", h=6),
                                )

                        if taps and pj == 0:
                            nc.sync.dma_start(t_qT[:], qTp[:].bitcast(F32))
                            nc.sync.dma_start(t_kT[:], kTp[:].bitcast(F32))
                            nc.sync.dma_start(t_v[:], vsl[:].bitcast(F32).rearrange("p a b c -> p (a b c)"))
                        # ---- attention for the two heads of this pair ----
                        for n in range(NBLK):
                            avs = [
                                ps.tile([D + 1, 512], F32, tag="av", bufs=2, name=f"av{hi}")
                                for hi in range(2)
                            ]
                            for sk in range(NSEQ):
                                sc = ps.tile([128, 2, 512], F32, tag="sc")
                                for hi in range(2):
                                    nc.tensor.matmul(
                                        sc[:, hi, :],
                                        kTp[
                                            hi * D : (hi + 1) * D,
                                            sk * 128 : (sk + 1) * 128,
                                        ],
                                        qTp[
                                            hi * D : (hi + 1) * D,
                                            n * 512 : (n + 1) * 512,
                                        ],
                                        start=True,
                                        stop=True,
                                    )
                                pt = ptp.tile([128, 2, 512], F32R, tag="pt")
                                nc.scalar.activation(
                                    pt[:], sc[:], EXP, bias=0.0, scale=0.125
                                )
                                if taps and pj == 0 and n == 0 and sk == 0:
                                    nc.sync.dma_start(t_pt[:], pt[:].bitcast(F32).rearrange("p a b -> p (a b)"))
                                for hi in range(2):
                                    nc.tensor.matmul(
                                        avs[hi][:],
                                        vsl[:, sk, 2 * pj + hi, :],
                                        pt[:, hi, :],
                                        start=(sk == 0),
                                        stop=(sk == NSEQ - 1),
                                    )
                            # drain av psum to SBUF right away (frees the psum
                            # slot for the next block) then normalize from SBUF
                            av_sbs = []
                            for hi in range(2):
                                av_sb = smp.tile([D + 1, 512], F32, tag="avsb", name=f"av_sb{hi}")
                                nc.vector.tensor_copy(av_sb[:], avs[hi][:])
                                av_sbs.append(av_sb)
                            if taps and pj == 0 and n == 0:
                                nc.sync.dma_start(t_av[:], av_sbs[0][:])
                            den = smp.tile([33, 512], F32, tag="den")
                            if den_init[0] < 2:
                                den_init[0] += 1
                                nc.vector.memset(den[:], 1.0)
                            for hi in range(2):
                                nc.vector.tensor_copy(
                                    den[32 * hi : 32 * hi + 1, :],
                                    av_sbs[hi][D : D + 1, :],
                                )
                            rec_sb = smp.tile([33, 512], F32, tag="rec")
                            nc.vector.reciprocal(rec_sb[:], den[:])
                            rec_r = smp.tile([33, 512], F32R, tag="recr")
                            nc.vector.tensor_copy(rec_r[:], rec_sb[:])
                            for hi in range(2):
                                bc = ps.tile([D, 512], F32, tag="bc", bufs=1)
                                nc.tensor.matmul(
                                    bc[:],
                                    ones_r[32 * hi : 32 * hi + 1, 0:D],
                                    rec_r[32 * hi : 32 * hi + 1, :],
                                    start=True,
                                    stop=True,
                                )
                                if taps and pj == 0 and n == 0 and hi == 0:
                                    nc.sync.dma_start(t_rc[:], rec_r[0:1, :].bitcast(F32))
                                nc.vector.tensor_mul(
                                    aoT[
                                        hi * D : (hi + 1) * D,
                                        pj,
                                        n * 512 : (n + 1) * 512,
                                    ],
                                    av_sbs[hi][0:D, :],
                                    bc[:],
                                )

            if taps:
                nc.sync.dma_start(t_ao[:], aoT[:].bitcast(F32).rearrange("p a b -> p (a b)"))
            # ---- projection: yT[768, S] = wp^T @ aoT (partial) ----
            yT_ap = yT[:].rearrange("(c p) s -> p c s", p=128)
            with tc.tile_pool(name="yt", bufs=6) as ytp:
                for n in range(NBLK):
                    for m in range(6):
                        pp = ps.tile([128, 512], F32, tag="qk", bufs=1)
                        for k in range(3):
                            nc.tensor.matmul(
                                pp[:],
                                wp_r[:, k, m * 128 : (m + 1) * 128],
                                aoT[:, k, n * 512 : (n + 1) * 512],
                                start=(k == 0),
                                stop=(k == 2),
                            )
                        yt_t = ytp.tile([128, 512], F32, tag="yT")
                        if m % 2 == 0:
                            nc.scalar.copy(yt_t[:], pp[:])
                        else:
                            nc.vector.tensor_copy(yt_t[:], pp[:])
                        nc.sync.dma_start(
                            yT_ap[:, m, n * 512 : (n + 1) * 512], yt_t[:]
                        )

    nc.finalize()
    return nc


_NC_CACHE = {}


def _get_nc(S, taps=False):
    key = (S, taps)
    if key not in _NC_CACHE:
        _NC_CACHE[key] = build_nc(S, taps)
    return _NC_CACHE[key]


def kernel(x, qkv_w, qkv_b, proj_w, proj_b, return_res=False, **run_kwargs):
    x = np.asarray(x, dtype=np.float32)
    qkv_w = np.asarray(qkv_w, dtype=np.float32)
    qkv_b = np.asarray(qkv_b, dtype=np.float32)
    proj_w = np.asarray(proj_w, dtype=np.float32)
    proj_b = np.asarray(proj_b, dtype=np.float32)
    B, S, _ = x.shape

    nc = _get_nc(S)
    in_maps = []
    for c in range(8):
        b, g = c // 2, c % 2
        qs = slice(384 * g, 384 * g + 384)
        ks = slice(768 + 384 * g, 768 + 384 * g + 384)
        vs = slice(1536 + 384 * g, 1536 + 384 * g + 384)
        in_maps.append(
            {
                "x": np.ascontiguousarray(x[b]),
                "wqk": np.ascontiguousarray(
                    np.concatenate([qkv_w[:, qs], qkv_w[:, ks]], axis=1)
                ),
                "wv": np.ascontiguousarray(qkv_w[:, vs]),
                "bqk": np.ascontiguousarray(
                    np.concatenate([qkv_b[qs], qkv_b[ks]])
                ),
                "bv": np.ascontiguousarray(qkv_b[vs]),
                "wp": np.ascontiguousarray(proj_w[384 * g : 384 * g + 384, :]),
            }
        )
    try:
        res = run_bass_kernel_spmd(
            nc, in_maps, core_ids=list(range(8)), **run_kwargs
        )
    except Exception:
        # transient NRT/device errors happen occasionally; retry once
        res = run_bass_kernel_spmd(
            nc, in_maps, core_ids=list(range(8)), **run_kwargs
        )
    out = np.empty((B, S, HID), np.float32)
    for b in range(B):
        yt = res.results[2 * b]["yT"] + res.results[2 * b + 1]["yT"]
        out[b] = yt.T + proj_b
    if return_res:
        return out, res
    return out



# revision 22
# speedup vs baseline: 1.1510x; 1.1510x over previous
"""MultiHeadAttention Trainium2 kernel (8 NeuronCores).

Sharding: core c -> (batch b = c//2, head-group g = c%2) of the 12 heads.
Each core computes attention for its 6 heads of one batch element and a
partial projection; the host sums the two head-group partials per batch
element and adds the effective proj bias (proj_b + bv @ proj_w; the v bias
is additive after softmax because attention rows sum to 1).

Per-core dataflow (bf16 datapath, fp8 DoubleRow scores):
  x bf16 [S,768] --PE-transpose--> xT bf16 [128,6,S]
  q/k psum f32 = wqk_bf16^T @ xT; DVE adds bias and converts to fp8 in a
    zero-padded DoubleRow layout q8/k8 [64|64 part, pair, 2, S]
  v bf16 seq-major vsl [sk, skpair, j, head, 65] (+ ones col for denom)
  scores[sk,sq] = DoubleRow fp8 matmul (contraction 64 + 64 zeros)
  pt = exp(scores/8): split ACT (exact) / Pool / DVE (Schraudolph bits)
  av[sq,2,65] += pt-chunk^T @ v    (bf16, psum accum; col 64 = denom)
  ao_n = av / denom (broadcast divide, DVE), PE-transpose to aoT [384,S]
  yT[768,S] = wp_bf16^T @ aoT      (partial projection, host sums pairs)
"""
import sys

sys.path.insert(0, "/opt/trn_rl_repo")

import numpy as np

import concourse.bass as bass
import concourse.mybir as mybir
import concourse.tile as tile
from concourse import bacc
from concourse.bass_utils import run_bass_kernel_spmd
from concourse.masks import make_identity

F32 = mybir.dt.float32
BF16 = mybir.dt.bfloat16
U16 = mybir.dt.uint16
FP8 = mybir.dt.float8e4
EXP = mybir.ActivationFunctionType.Exp
ADD = mybir.AluOpType.add
MULT = mybir.AluOpType.mult
DIV = mybir.AluOpType.divide
DR = mybir.MatmulPerfMode.DoubleRow

HID = 768
D = 64  # head dim
LHEADS = 6  # heads per core
PAIRS = 3

LOG2E = 1.4426950408889634
# Schraudolph-in-bf16-bits: n = x*0.125*128*log2e + (16256 + c); floor via
# the executor's f32->u16 cast. c=-6.85 calibrated for min rms vs exp().
SCHR_MUL = 0.125 * 128.0 * LOG2E
SCHR_ADD = 16256.0 - 6.85

# exp engine split: per block of 16 exp units (1024 rows each).
# GPSIMD (Pool) cannot access PSUM on TRN2, so only ACT and DVE apply.
EXP_PATTERN = (
    "act", "dve", "act", "act", "dve", "act", "act", "dve",
    "act", "dve", "act", "act", "dve", "act", "act", "act",
)


def build_nc(S: int, taps: bool = False):
    nc = bacc.Bacc("TRN2", target_bir_lowering=False, debug=False)
    NSEQ = S // 128  # seq chunks of 128
    NBLK = S // 512  # seq blocks of 512
    NPAIR = NSEQ // 2  # sk chunk pairs
    XG = 4  # x DMA chunk groups
    NXG = NSEQ // XG

    x = nc.dram_tensor("x", [S, HID], BF16, kind="ExternalInput")
    wqk = nc.dram_tensor("wqk", [HID, 768], BF16, kind="ExternalInput")
    wv = nc.dram_tensor("wv", [HID, 384], BF16, kind="ExternalInput")
    bqk = nc.dram_tensor("bqk", [768], F32, kind="ExternalInput")
    wp = nc.dram_tensor("wp", [384, HID], BF16, kind="ExternalInput")
    yT = nc.dram_tensor("yT", [HID, S], F32, kind="ExternalOutput")
    if taps:
        t_xT = nc.dram_tensor("t_xT", [128, 6 * S], BF16, kind="ExternalOutput")
        t_q8 = nc.dram_tensor("t_q8", [128, 2 * S], FP8, kind="ExternalOutput")
        t_k8 = nc.dram_tensor("t_k8", [128, 2 * S], FP8, kind="ExternalOutput")
        t_v = nc.dram_tensor("t_v", [128, NSEQ // 2 * 2 * 6 * (D + 1)], BF16, kind="ExternalOutput")
        t_pt = nc.dram_tensor("t_pt", [128, 16 * 2 * 512], BF16, kind="ExternalOutput")
        t_ao = nc.dram_tensor("t_ao", [128, PAIRS * S], BF16, kind="ExternalOutput")
        t_aon = nc.dram_tensor("t_aon", [128, 2 * D], BF16, kind="ExternalOutput")
        t_qp = nc.dram_tensor("t_qp", [128, 2, 512], F32, kind="ExternalOutput")

    with tile.TileContext(nc) as tc:
        with (
            tc.tile_pool(name="const", bufs=1) as cp,
            tc.tile_pool(name="wts", bufs=1) as wpool,
            tc.tile_pool(name="qk8", bufs=1) as qk8p,
            tc.tile_pool(name="ao", bufs=1) as aop,
            tc.tile_pool(name="ps", bufs=1, space="PSUM") as ps,
        ):
            identf = cp.tile([128, 128], F32, tag="identf")
            make_identity(nc, identf[:])
            ident = cp.tile([128, 128], BF16, tag="ident")
            nc.vector.tensor_copy(ident[:], identf[:])
            # q/k bias, feature-major [128, 6] (chunk c: c<3 q, c>=3 k)
            bqk_sb = cp.tile([128, 6], F32, tag="bqk")
            nc.sync.dma_start(bqk_sb[:], bqk[:].rearrange("(c p) -> p c", p=128))
            # load the exp ACT table off the critical path
            warm = cp.tile([1, 16], F32, tag="warm")
            nc.vector.memset(warm[:], 1.0)
            nc.scalar.activation(warm[:], warm[:], EXP, bias=0.0, scale=0.0)

            # fp8 q/k, zero-padded DoubleRow layout, one tile per pair:
            # [128, 2, S]; partition half = head-of-pair, dim1 = j (j=1
            # stays zero) so [64*hi:64*hi+64, :, a:b] is a DR operand.
            q8_tiles = {}
            k8_tiles = {}

            def get_qk8(p):
                if p not in q8_tiles:
                    q8_p = qk8p.tile([128, 2, S], FP8, tag="q8", bufs=3,
                                     name=f"q8_{p}")
                    k8_p = qk8p.tile([128, 2, S], FP8, tag="k8", bufs=3,
                                     name=f"k8_{p}")
                    nc.vector.memset(q8_p[:, 1, :], 0.0)
                    nc.vector.memset(k8_p[:, 1, :], 0.0)
                    q8_tiles[p] = q8_p
                    k8_tiles[p] = k8_p
                return q8_tiles[p], k8_tiles[p]

            aoT = aop.tile([128, PAIRS, S], BF16, tag="aoT")

            with tc.tile_pool(name="xT", bufs=1) as xtp, \
                 tc.tile_pool(name="vv", bufs=1) as vvp:
                xT = xtp.tile([128, 6, S], BF16, tag="xT")
                # v seq-major [sk, skpair, j, head, 65]; col 64 = ones
                vsl = vvp.tile([128, NPAIR, 2, LHEADS, D + 1], BF16, tag="v")
                nc.vector.memset(vsl[:, :, :, :, D : D + 1], 1.0)

                # --- x DMA, weights, transposes ---
                with tc.tile_pool(name="xin", bufs=1) as xin:
                    x_ap = x[:].rearrange("(n p) d -> p n d", p=128)
                    x_ts = []
                    for g in range(XG):
                        x_t = xin.tile([128, NXG, HID], BF16, tag=f"x{g}", name=f"x_t{g}")
                        x_ts.append(x_t)
                    nc.sync.dma_start(x_ts[0][:], x_ap[:, 0:NXG, :])
                    wqk_r = wpool.tile([128, 6, 768], BF16, tag="wqkr")
                    wqk_ap = wqk[:].rearrange("(c p) f -> p c f", p=128)
                    for kc in range(6):
                        nc.sync.dma_start(
                            wqk_r[:, kc : kc + 1, :], wqk_ap[:, kc : kc + 1, :]
                        )
                    for g in range(1, XG):
                        nc.sync.dma_start(
                            x_ts[g][:], x_ap[:, g * NXG : (g + 1) * NXG, :]
                        )
                    wv_r = wpool.tile([128, 6, 384], BF16, tag="wvr")
                    nc.sync.dma_start(
                        wv_r[:], wv[:].rearrange("(c p) f -> p c f", p=128)
                    )
                    wp_r = wpool.tile([128, 3, HID], BF16, tag="wpr")
                    nc.sync.dma_start(
                        wp_r[:], wp[:].rearrange("(c p) f -> p c f", p=128)
                    )

                    # transposes: per x-group, per hid-chunk j, 4 seq chunks
                    # into one psum tile, then one contiguous copy
                    for g in range(XG):
                        for j in range(6):
                            tp = ps.tile([128, NXG, 128], BF16, tag="sc", bufs=2)
                            for i in range(NXG):
                                nc.tensor.transpose(
                                    tp[:, i, :],
                                    x_ts[g][:, i, j * 128 : (j + 1) * 128],
                                    ident[:],
                                )
                            dst = xT[:, j, g * NXG * 128 : (g + 1) * NXG * 128]
                            if (g * 6 + j) % 2 == 0:
                                nc.scalar.copy(dst, tp[:])
                            else:
                                nc.vector.tensor_copy(dst, tp[:])

                smp_holder = [None]

                def emit_qk(p, n):
                    """q+k psum for pair p, block n -> fp8 padded layout."""
                    qp = ps.tile([128, 2, 512], F32, tag="qkp", bufs=1)
                    for qk_i, wcol in ((0, p), (1, 3 + p)):
                        for kc in range(6):
                            nc.tensor.matmul(
                                qp[:, qk_i, :],
                                wqk_r[:, kc, wcol * 128 : (wcol + 1) * 128],
                                xT[:, kc, n * 512 : (n + 1) * 512],
                                start=(kc == 0),
                                stop=(kc == 5),
                                skip_group_check=True,
                            )
                    sl = slice(n * 512, (n + 1) * 512)
                    if taps and p == 0 and n == 0:
                        qp_sb = cp.tile([128, 2, 512], F32, tag="qptap", name="qp_sb")
                        nc.vector.tensor_copy(qp_sb[:], qp[:])
                        nc.sync.dma_start(t_qp[:], qp_sb[:])
                    q8_p, k8_p = get_qk8(p)
                    for dst8, qk_i, wcol in ((q8_p, 0, p), (k8_p, 1, 3 + p)):
                        for hi in range(2):
                            nc.vector.tensor_scalar(
                                dst8[64 * hi : 64 * hi + 64, 0, sl],
                                qp[64 * hi : 64 * hi + 64, qk_i, :],
                                bqk_sb[64 * hi : 64 * hi + 64, wcol : wcol + 1],
                                None,
                                ADD,
                            )

                def emit_v(i):
                    """v for seq chunk i, all 6 heads, seq-major, bias-free."""
                    vp = ps.tile([128, 2, 512], F32, tag="qkp", bufs=1)
                    for kc in range(6):
                        nc.tensor.matmul(
                            vp[:, 0, 0:384],
                            xT[:, kc, i * 128 : (i + 1) * 128],
                            wv_r[:, kc, :],
                            start=(kc == 0),
                            stop=(kc == 5),
                            skip_group_check=True,
                        )
                    nc.vector.tensor_copy(
                        vsl[:, i // 2, i % 2, :, 0:D],
                        vp[:, 0, 0:384].rearrange("p (h d) -> p h d", h=6),
                    )

                # qk for pair 0, all blocks, before attention starts
                for n in range(NBLK):
                    emit_qk(0, n)

                with (
                    tc.tile_pool(name="pt", bufs=1) as ptp,
                    tc.tile_pool(name="sm", bufs=1) as smp,
                ):
                    smp_holder[0] = smp
                    exp_idx = [0]

                    def emit_scores(p, n, pt):
                        q8_p, k8_p = get_qk8(p)
                        for hi in range(2):
                            for i in range(NPAIR):
                                sc = ps.tile([128, 2, 512], F32, tag="sc", bufs=2)
                                for j in range(2):
                                    sk = 2 * i + j
                                    nc.tensor.matmul(
                                        sc[:, j, :],
                                        k8_p[64 * hi : 64 * hi + 64, :,
                                             sk * 128 : (sk + 1) * 128],
                                        q8_p[64 * hi : 64 * hi + 64, :,
                                             n * 512 : (n + 1) * 512],
                                        start=True,
                                        stop=True,
                                        perf_mode=DR,
                                    )
                                dst = pt[:, 8 * hi + i, :, :]
                                eng = EXP_PATTERN[exp_idx[0] % len(EXP_PATTERN)]
                                exp_idx[0] += 1
                                if eng == "act":
                                    nc.scalar.activation(
                                        dst, sc[:], EXP, bias=0.0, scale=0.125
                                    )
                                else:
                                    for j in range(2):
                                        nc.vector.tensor_scalar(
                                            pt[:, 8 * hi + i, j, :].bitcast(U16),
                                            sc[:, j, :],
                                            SCHR_MUL, SCHR_ADD, MULT, ADD,
                                        )

                    ao_ns = {}

                    def emit_av(p, n, pt):
                        for c in range(4):
                            av = ps.tile([128, 2, D + 1], F32, tag="av", bufs=2)
                            first = True
                            for hi in range(2):
                                for i in range(NPAIR):
                                    for j in range(2):
                                        nc.tensor.matmul(
                                            av[:, hi, :],
                                            pt[:, 8 * hi + i, j,
                                               c * 128 : (c + 1) * 128],
                                            vsl[:, i, j, 2 * p + hi, :],
                                            start=first,
                                            stop=(hi == 1 and i == NPAIR - 1
                                                  and j == 1),
                                            skip_group_check=True,
                                        )
                                        first = False
                            rec = smp.tile([128, 2], F32, tag="rec", bufs=4)
                            nc.vector.reciprocal(rec[:], av[:, :, D])
                            ao_n = smp.tile([128, 2, D], BF16, tag="aon", bufs=32,
                                            name=f"ao_n_{p}_{n}_{c}")
                            nc.vector.tensor_tensor(
                                ao_n[:],
                                av[:, :, 0:D],
                                rec[:].unsqueeze(2).broadcast_to([128, 2, D]),
                                MULT,
                            )
                            ao_ns[(p, n, c)] = ao_n
                            if taps and p == 0 and n == 0 and c == 0:
                                tap(t_aon, ao_n[:].rearrange("p a b -> p (a b)"))

                    def emit_aoT(p):
                        # transpose the pair's normalized outputs into aoT
                        for n in range(NBLK):
                            tp2 = ps.tile([128, 4, 128], BF16, tag="sc",
                                          bufs=2, name=f"tp2_{p}_{n}")
                            for c in range(4):
                                nc.tensor.transpose(
                                    tp2[:, c, :],
                                    ao_ns.pop((p, n, c))[:].rearrange(
                                        "p h d -> p (h d)"),
                                    ident[:],
                                )
                            nc.vector.tensor_copy(
                                aoT[:, p, n * 512 : (n + 1) * 512],
                                tp2[:].rearrange("p a b -> p (a b)"),
                            )

                    if taps:
                        def tap(dram, ap):
                            nc.sync.dma_start(dram[:], ap)
                    pt_tiles = {}
                    prev = None
                    for p in range(PAIRS):
                        for n in range(NBLK):
                            pt = ptp.tile([128, 16, 2, 512], BF16, tag="pt",
                                          bufs=2, name=f"pt_{p}_{n}")
                            pt_tiles[(p, n)] = pt
                            emit_scores(p, n, pt)
                            if taps and p == 0 and n == 0:
                                tap(t_xT, xT[:].rearrange("p a b -> p (a b)"))
                                tap(t_q8, q8_tiles[0][:].rearrange("p b c -> p (b c)"))
                                tap(t_k8, k8_tiles[0][:].rearrange("p b c -> p (b c)"))
                                tap(t_v, vsl[:].rearrange("p a b c d -> p (a b c d)"))
                                tap(t_pt, pt[:].rearrange("p a b c -> p (a b c)"))
                            if p == 0 and n < 2:
                                for i in range(8 * n, 8 * n + 8):
                                    emit_v(i)
                            if p < PAIRS - 1:
                                emit_qk(p + 1, n)
                            if prev is not None:
                                emit_av(*prev)
                                if prev[1] == NBLK - 1:
                                    emit_aoT(prev[0])
                            prev = (p, n, pt)
                    emit_av(*prev)
                    emit_aoT(prev[0])
                    if taps:
                        tap(t_ao, aoT[:].rearrange("p a b -> p (a b)"))

            # ---- projection: yT[768, S] = wp^T @ aoT (partial) ----
            yT_ap = yT[:].rearrange("(c p) s -> p c s", p=128)
            with tc.tile_pool(name="yt", bufs=6) as ytp:
                for n in range(NBLK):
                    for m in range(6):
                        pp = ps.tile([128, 2, 512], F32, tag="sc", bufs=2)
                        for kc in range(3):
                            nc.tensor.matmul(
                                pp[:, 0, :],
                                wp_r[:, kc, m * 128 : (m + 1) * 128],
                                aoT[:, kc, n * 512 : (n + 1) * 512],
                                start=(kc == 0),
                                stop=(kc == 2),
                                skip_group_check=True,
                            )
                        yt_t = ytp.tile([128, 512], F32, tag="yT")
                        if m % 2 == 0:
                            nc.scalar.copy(yt_t[:], pp[:, 0, :])
                        else:
                            nc.vector.tensor_copy(yt_t[:], pp[:, 0, :])
                        nc.sync.dma_start(
                            yT_ap[:, m, n * 512 : (n + 1) * 512], yt_t[:]
                        )

    nc.finalize()
    return nc


_NC_CACHE = {}


def _get_nc(S):
    if S not in _NC_CACHE:
        _NC_CACHE[S] = build_nc(S)
    return _NC_CACHE[S]


def kernel(x, qkv_w, qkv_b, proj_w, proj_b, return_res=False, **run_kwargs):
    import ml_dtypes

    x = np.asarray(x, dtype=np.float32)
    qkv_w = np.asarray(qkv_w, dtype=np.float32)
    qkv_b = np.asarray(qkv_b, dtype=np.float32)
    proj_w = np.asarray(proj_w, dtype=np.float32)
    proj_b = np.asarray(proj_b, dtype=np.float32)
    B, S, _ = x.shape

    nc = _get_nc(S)
    bf = ml_dtypes.bfloat16
    x_bf = x.astype(bf)
    in_maps = []
    for c in range(8):
        b, g = c // 2, c % 2
        qs = slice(384 * g, 384 * g + 384)
        ks = slice(768 + 384 * g, 768 + 384 * g + 384)
        vs = slice(1536 + 384 * g, 1536 + 384 * g + 384)
        in_maps.append(
            {
                "x": np.ascontiguousarray(x_bf[b]).view(np.uint16),
                "wqk": np.ascontiguousarray(
                    np.concatenate([qkv_w[:, qs], qkv_w[:, ks]], axis=1).astype(bf)
                ).view(np.uint16),
                "wv": np.ascontiguousarray(qkv_w[:, vs].astype(bf)).view(np.uint16),
                "bqk": np.ascontiguousarray(
                    np.concatenate([qkv_b[qs], qkv_b[ks]])
                ),
                "wp": np.ascontiguousarray(
                    proj_w[384 * g : 384 * g + 384, :].astype(bf)
                ).view(np.uint16),
            }
        )
    try:
        res = run_bass_kernel_spmd(
            nc, in_maps, core_ids=list(range(8)), **run_kwargs
        )
    except Exception:
        # transient NRT/device errors happen occasionally; retry once
        res = run_bass_kernel_spmd(
            nc, in_maps, core_ids=list(range(8)), **run_kwargs
        )
    # effective bias: the v bias passes through softmax additively
    b_eff = (proj_b.astype(np.float64)
             + qkv_b[1536:].astype(np.float64) @ proj_w.astype(np.float64)
             ).astype(np.float32)
    out = np.empty((B, S, HID), np.float32)
    for b in range(B):
        yt = res.results[2 * b]["yT"] + res.results[2 * b + 1]["yT"]
        out[b] = yt.T + b_eff
    if return_res:
        return out, res
    return out


# revision 32
# speedup vs baseline: 1.2702x; 1.1035x over previous
"""MultiHeadAttention Trainium2 kernel (8 NeuronCores).

Sharding: core c -> (batch b = c//2, head-group g = c%2) of the 12 heads.
Each core computes attention for its 6 heads of one batch element and a
partial projection; the host sums the two head-group partials per batch
element and adds the effective proj bias (proj_b + bv @ proj_w; the v bias
is additive after softmax because attention rows sum to 1).

Per-core dataflow (bf16 datapath, fp8 DoubleRow scores):
  x bf16 [S,768] --PE-transpose--> xT bf16 [128,6,S]
  q/k psum f32 = wqk_bf16^T @ xT; DVE adds bias and converts to fp8 in a
    zero-padded DoubleRow layout q8/k8 [64|64 part, pair, 2, S]
  v bf16 seq-major vsl [sk, skpair, j, head, 65] (+ ones col for denom)
  scores[sk,sq] = DoubleRow fp8 matmul (contraction 64 + 64 zeros)
  pt = exp(scores/8): split ACT (exact) / Pool / DVE (Schraudolph bits)
  av[sq,2,65] += pt-chunk^T @ v    (bf16, psum accum; col 64 = denom)
  ao_n = av / denom (broadcast divide, DVE), PE-transpose to aoT [384,S]
  yT[768,S] = wp_bf16^T @ aoT      (partial projection, host sums pairs)
"""
import sys

sys.path.insert(0, "/opt/trn_rl_repo")

import numpy as np

import concourse.bass as bass
import concourse.mybir as mybir
import concourse.tile as tile
from concourse import bacc
from concourse.bass_utils import run_bass_kernel_spmd
from concourse.masks import make_identity

F32 = mybir.dt.float32
BF16 = mybir.dt.bfloat16
U16 = mybir.dt.uint16
FP8 = mybir.dt.float8e4
EXP = mybir.ActivationFunctionType.Exp
COPY_FN = mybir.ActivationFunctionType.Identity
ADD = mybir.AluOpType.add
MULT = mybir.AluOpType.mult
DIV = mybir.AluOpType.divide
DR = mybir.MatmulPerfMode.DoubleRow

HID = 768
D = 64  # head dim
LHEADS = 6  # heads per core
PAIRS = 3

LOG2E = 1.4426950408889634
# Schraudolph-in-bf16-bits: n = x*0.125*128*log2e + (16256 + c); floor via
# the executor's f32->u16 cast. c=-6.85 calibrated for min rms vs exp().
SCHR_MUL = 0.125 * 128.0 * LOG2E
SCHR_ADD = 16256.0 - 6.85

# exp engine split: per block of 16 exp units (1024 rows each).
# GPSIMD (Pool) cannot access PSUM on TRN2, so only ACT and DVE apply.
EXP_PATTERN = (
    "act", "dve", "act", "act", "dve", "act", "dve", "act",
    "act", "dve", "act", "dve", "act", "act", "dve", "act",
)


def build_nc(S: int, taps: bool = False):
    nc = bacc.Bacc("TRN2", target_bir_lowering=False, debug=False)
    NSEQ = S // 128  # seq chunks of 128
    NBLK = S // 512  # seq blocks of 512
    NPAIR = NSEQ // 2  # sk chunk pairs
    XG = 4  # x DMA chunk groups
    NXG = NSEQ // XG

    x = nc.dram_tensor("x", [S, HID], BF16, kind="ExternalInput")
    wqk = nc.dram_tensor("wqk", [HID, 768], BF16, kind="ExternalInput")
    wv = nc.dram_tensor("wv", [HID, 384], BF16, kind="ExternalInput")
    bqk = nc.dram_tensor("bqk", [768], F32, kind="ExternalInput")
    wp = nc.dram_tensor("wp", [384, HID], BF16, kind="ExternalInput")
    z8 = nc.dram_tensor("z8", [128, 2048], FP8, kind="ExternalInput")
    yT = nc.dram_tensor("yT", [HID, S], F32, kind="ExternalOutput")
    if taps:
        t_xT = nc.dram_tensor("t_xT", [128, 6 * S], BF16, kind="ExternalOutput")
        t_q8 = nc.dram_tensor("t_q8", [128, 2 * S], FP8, kind="ExternalOutput")
        t_k8 = nc.dram_tensor("t_k8", [128, 2 * S], FP8, kind="ExternalOutput")
        t_v = nc.dram_tensor("t_v", [128, NSEQ // 2 * 2 * 6 * (D + 1)], BF16, kind="ExternalOutput")
        t_pt = nc.dram_tensor("t_pt", [128, 16 * 2 * 512], BF16, kind="ExternalOutput")
        t_ao = nc.dram_tensor("t_ao", [128, PAIRS * S], BF16, kind="ExternalOutput")
        t_aon = nc.dram_tensor("t_aon", [128, 2 * D], BF16, kind="ExternalOutput")
        t_qp = nc.dram_tensor("t_qp", [128, 2, 512], F32, kind="ExternalOutput")

    with tile.TileContext(nc) as tc:
        with (
            tc.tile_pool(name="const", bufs=1) as cp,
            tc.tile_pool(name="wts", bufs=1) as wpool,
            tc.tile_pool(name="qk8", bufs=1) as qk8p,
            tc.tile_pool(name="ao", bufs=1) as aop,
            tc.tile_pool(name="ps", bufs=1, space="PSUM") as ps,
        ):
            identf = cp.tile([128, 128], F32, tag="identf")
            make_identity(nc, identf[:])
            ident = cp.tile([128, 128], BF16, tag="ident")
            nc.vector.tensor_copy(ident[:], identf[:])
            # q/k bias, feature-major [128, 6] (chunk c: c<3 q, c>=3 k)
            bqk_sb = cp.tile([128, 6], F32, tag="bqk")
            nc.sync.dma_start(bqk_sb[:], bqk[:].rearrange("(c p) -> p c", p=128))
            # load the exp ACT table off the critical path
            warm = cp.tile([1, 16], F32, tag="warm")
            nc.vector.memset(warm[:], 1.0)
            nc.scalar.activation(warm[:], warm[:], EXP, bias=0.0, scale=0.0)

            # fp8 q/k, zero-padded DoubleRow layout, one tile per pair:
            # [128, 2, S]; partition half = head-of-pair, dim1 = j (j=1
            # stays zero) so [64*hi:64*hi+64, :, a:b] is a DR operand.
            q8_tiles = {}
            k8_tiles = {}

            def get_qk8(p):
                if p not in q8_tiles:
                    q8_p = qk8p.tile([128, 2, S], FP8, tag="q8", bufs=3,
                                     name=f"q8_{p}")
                    k8_p = qk8p.tile([128, 2, S], FP8, tag="k8", bufs=3,
                                     name=f"k8_{p}")
                    nc.sync.dma_start(q8_p[:, 1, :], z8[:])
                    nc.sync.dma_start(k8_p[:, 1, :], z8[:])
                    q8_tiles[p] = q8_p
                    k8_tiles[p] = k8_p
                return q8_tiles[p], k8_tiles[p]

            aoT = aop.tile([128, PAIRS, S], BF16, tag="aoT")

            with tc.tile_pool(name="xT", bufs=1) as xtp, \
                 tc.tile_pool(name="vv", bufs=1) as vvp:
                xT = xtp.tile([128, 6, S], BF16, tag="xT")
                # v seq-major [sk, skpair, j, head, 65]; col 64 = ones
                vsl = vvp.tile([128, NPAIR, 2, LHEADS, D + 1], BF16, tag="v")
                nc.vector.memset(vsl[:, :, :, :, D : D + 1], 1.0)

                # --- x DMA, weights, transposes ---
                with tc.tile_pool(name="xin", bufs=1) as xin:
                    x_ap = x[:].rearrange("(n p) d -> p n d", p=128)
                    x_ts = []
                    for g in range(XG):
                        x_t = xin.tile([128, NXG, HID], BF16, tag=f"x{g}", name=f"x_t{g}")
                        x_ts.append(x_t)
                    nc.sync.dma_start(x_ts[0][:], x_ap[:, 0:NXG, :])
                    wqk_r = wpool.tile([128, 6, 768], BF16, tag="wqkr")
                    wqk_ap = wqk[:].rearrange("(c p) f -> p c f", p=128)
                    for kc in range(6):
                        nc.sync.dma_start(
                            wqk_r[:, kc : kc + 1, :], wqk_ap[:, kc : kc + 1, :]
                        )
                    for g in range(1, XG):
                        nc.sync.dma_start(
                            x_ts[g][:], x_ap[:, g * NXG : (g + 1) * NXG, :]
                        )
                    wv_r = wpool.tile([128, 6, 384], BF16, tag="wvr")
                    nc.sync.dma_start(
                        wv_r[:], wv[:].rearrange("(c p) f -> p c f", p=128)
                    )
                    wp_r = wpool.tile([128, 3, HID], BF16, tag="wpr")
                    nc.sync.dma_start(
                        wp_r[:], wp[:].rearrange("(c p) f -> p c f", p=128)
                    )

                    # transposes: per x-group, per hid-chunk j, 4 seq chunks
                    # into one psum tile, then one contiguous copy
                    for g in range(XG):
                        for j in range(6):
                            tp = ps.tile([128, NXG, 128], BF16, tag="sc", bufs=3)
                            for i in range(NXG):
                                nc.tensor.transpose(
                                    tp[:, i, :],
                                    x_ts[g][:, i, j * 128 : (j + 1) * 128],
                                    ident[:],
                                )
                            dst = xT[:, j, g * NXG * 128 : (g + 1) * NXG * 128]
                            if (g * 6 + j) % 2 == 0:
                                nc.scalar.copy(dst, tp[:])
                            else:
                                nc.vector.tensor_copy(dst, tp[:])

                smp_holder = [None]

                def emit_qk(p, n):
                    """q+k psum for pair p, block n -> fp8 padded layout."""
                    qp = ps.tile([128, 2, 512], F32, tag="sc", bufs=3)
                    for qk_i, wcol in ((0, p), (1, 3 + p)):
                        for kc in range(6):
                            nc.tensor.matmul(
                                qp[:, qk_i, :],
                                wqk_r[:, kc, wcol * 128 : (wcol + 1) * 128],
                                xT[:, kc, n * 512 : (n + 1) * 512],
                                start=(kc == 0),
                                stop=(kc == 5),
                                skip_group_check=True,
                            )
                    sl = slice(n * 512, (n + 1) * 512)
                    if taps and p == 0 and n == 0:
                        qp_sb = cp.tile([128, 2, 512], F32, tag="qptap", name="qp_sb")
                        nc.vector.tensor_copy(qp_sb[:], qp[:])
                        nc.sync.dma_start(t_qp[:], qp_sb[:])
                    q8_p, k8_p = get_qk8(p)
                    for dst8, qk_i, wcol in ((q8_p, 0, p), (k8_p, 1, 3 + p)):
                        for hi in range(2):
                            if (qk_i + hi) % 2 == 0:
                                nc.vector.tensor_scalar(
                                    dst8[64 * hi : 64 * hi + 64, 0, sl],
                                    qp[64 * hi : 64 * hi + 64, qk_i, :],
                                    bqk_sb[64 * hi : 64 * hi + 64, wcol : wcol + 1],
                                    None,
                                    ADD,
                                )
                            else:
                                nc.scalar.activation(
                                    dst8[64 * hi : 64 * hi + 64, 0, sl],
                                    qp[64 * hi : 64 * hi + 64, qk_i, :],
                                    COPY_FN,
                                    bias=bqk_sb[64 * hi : 64 * hi + 64, wcol : wcol + 1],
                                    scale=1.0,
                                )

                def emit_v(i):
                    """v for seq chunk i, all 6 heads, seq-major, bias-free."""
                    vp = ps.tile([128, 2, 512], F32, tag="sc", bufs=3)
                    for kc in range(6):
                        nc.tensor.matmul(
                            vp[:, 0, 0:384],
                            xT[:, kc, i * 128 : (i + 1) * 128],
                            wv_r[:, kc, :],
                            start=(kc == 0),
                            stop=(kc == 5),
                            skip_group_check=True,
                        )
                    nc.vector.tensor_copy(
                        vsl[:, i // 2, i % 2, :, 0:D],
                        vp[:, 0, 0:384].rearrange("p (h d) -> p h d", h=6),
                    )

                with (
                    tc.tile_pool(name="pt", bufs=1) as ptp,
                    tc.tile_pool(name="sm", bufs=1) as smp,
                ):
                    smp_holder[0] = smp
                    exp_idx = [0]

                    def emit_score_unit(p, n, pt, hi, i):
                        q8_p, k8_p = get_qk8(p)
                        sc = ps.tile([128, 2, 512], F32, tag="sc", bufs=3,
                                     name="sc")
                        for j in range(2):
                            sk = 2 * i + j
                            nc.tensor.matmul(
                                sc[:, j, :],
                                k8_p[64 * hi : 64 * hi + 64, :,
                                     sk * 128 : (sk + 1) * 128],
                                q8_p[64 * hi : 64 * hi + 64, :,
                                     n * 512 : (n + 1) * 512],
                                start=True,
                                stop=True,
                                perf_mode=DR,
                            )
                        dst = pt[:, 8 * hi + i, :, :]
                        eng = EXP_PATTERN[exp_idx[0] % len(EXP_PATTERN)]
                        exp_idx[0] += 1
                        if eng == "act":
                            nc.scalar.activation(
                                dst, sc[:], EXP, bias=0.0, scale=0.125
                            )
                        else:
                            nc.vector.tensor_scalar(
                                dst.bitcast(U16),
                                sc[:],
                                SCHR_MUL, SCHR_ADD, MULT, ADD,
                            )

                    def emit_scores(p, n, pt, hook=None):
                        u = 0
                        for hi in range(2):
                            for i in range(NPAIR):
                                emit_score_unit(p, n, pt, hi, i)
                                if hook is not None and u % 4 == 3:
                                    hook(u // 4)
                                u += 1

                    ao_ns = {}

                    def emit_av_chunk(p, n, pt, c):
                            av = ps.tile([128, 2, D + 1], F32, tag="av", bufs=2)
                            first = True
                            for hi in range(2):
                                for i in range(NPAIR):
                                    for j in range(2):
                                        nc.tensor.matmul(
                                            av[:, hi, :],
                                            pt[:, 8 * hi + i, j,
                                               c * 128 : (c + 1) * 128],
                                            vsl[:, i, j, 2 * p + hi, :],
                                            start=first,
                                            stop=(hi == 1 and i == NPAIR - 1
                                                  and j == 1),
                                            skip_group_check=True,
                                        )
                                        first = False
                            rec = smp.tile([128, 2], F32, tag="rec", bufs=4)
                            nc.vector.reciprocal(rec[:], av[:, :, D])
                            ao_n = smp.tile([128, 2, D], BF16, tag="aon", bufs=32,
                                            name=f"ao_n_{p}_{n}_{c}")
                            nc.vector.tensor_tensor(
                                ao_n[:],
                                av[:, :, 0:D],
                                rec[:].unsqueeze(2).broadcast_to([128, 2, D]),
                                MULT,
                            )
                            ao_ns[(p, n, c)] = ao_n

                    def emit_av(p, n, pt):
                        for c in range(4):
                            emit_av_chunk(p, n, pt, c)

                    def emit_aoT(p):
                        # transpose the pair's normalized outputs into aoT
                        for n in range(NBLK):
                            tp2 = ps.tile([128, 4, 128], BF16, tag="sc",
                                          bufs=3, name=f"tp2_{p}_{n}")
                            for c in range(4):
                                nc.tensor.transpose(
                                    tp2[:, c, :],
                                    ao_ns.pop((p, n, c))[:].rearrange(
                                        "p h d -> p (h d)"),
                                    ident[:],
                                )
                            nc.vector.tensor_copy(
                                aoT[:, p, n * 512 : (n + 1) * 512],
                                tp2[:].rearrange("p a b -> p (a b)"),
                            )

                    if taps:
                        def tap(dram, ap):
                            nc.sync.dma_start(dram[:], ap)
                    pt_tiles = {}
                    prev = None
                    for p in range(PAIRS):
                        for n in range(NBLK):
                            pt = ptp.tile([128, 16, 2, 512], BF16, tag="pt",
                                          bufs=2, name=f"pt_{p}_{n}")
                            pt_tiles[(p, n)] = pt
                            # interleave prev block's AV chunks into this
                            # block's score stream (safe once v is complete)
                            hook = None
                            if prev is not None and not (p == 0 and n <= 1):
                                pp_, pn_, ppt_ = prev
                                hook = lambda c: emit_av_chunk(pp_, pn_, ppt_, c)
                            if p == 0 and n == 0:
                                # k for the whole pair arrives per qk block;
                                # emit score units as their sk range lands
                                for kb in range(NBLK):
                                    emit_qk(0, kb)
                                    for i in (2 * kb, 2 * kb + 1):
                                        for hi in range(2):
                                            emit_score_unit(0, 0, pt, hi, i)
                            else:
                                emit_scores(p, n, pt, hook=hook)
                            if taps and p == 0 and n == 0:
                                tap(t_xT, xT[:].rearrange("p a b -> p (a b)"))
                                tap(t_q8, q8_tiles[0][:].rearrange("p b c -> p (b c)"))
                                tap(t_k8, k8_tiles[0][:].rearrange("p b c -> p (b c)"))
                                tap(t_v, vsl[:].rearrange("p a b c d -> p (a b c d)"))
                                tap(t_pt, pt[:].rearrange("p a b c -> p (a b c)"))
                            if p == 0 and n < 2:
                                for i in range(8 * n, 8 * n + 8):
                                    emit_v(i)
                            if p < PAIRS - 1:
                                emit_qk(p + 1, n)
                            if prev is not None and hook is None:
                                emit_av(*prev)
                            if prev is not None and prev[1] == NBLK - 1:
                                emit_aoT(prev[0])
                            prev = (p, n, pt)
                    emit_av(*prev)
                    emit_aoT(prev[0])
                    if taps:
                        tap(t_ao, aoT[:].rearrange("p a b -> p (a b)"))

            # ---- projection: yT[768, S] = wp^T @ aoT (partial) ----
            yT_ap = yT[:].rearrange("(c p) s -> p c s", p=128)
            with tc.tile_pool(name="yt", bufs=6) as ytp:
                for n in range(NBLK):
                    for m in range(6):
                        pp = ps.tile([128, 2, 512], F32, tag="sc", bufs=3)
                        for kc in range(3):
                            nc.tensor.matmul(
                                pp[:, 0, :],
                                wp_r[:, kc, m * 128 : (m + 1) * 128],
                                aoT[:, kc, n * 512 : (n + 1) * 512],
                                start=(kc == 0),
                                stop=(kc == 2),
                                skip_group_check=True,
                            )
                        yt_t = ytp.tile([128, 512], F32, tag="yT")
                        if m % 2 == 0:
                            nc.scalar.copy(yt_t[:], pp[:, 0, :])
                        else:
                            nc.vector.tensor_copy(yt_t[:], pp[:, 0, :])
                        nc.sync.dma_start(
                            yT_ap[:, m, n * 512 : (n + 1) * 512], yt_t[:]
                        )

    nc.finalize()
    return nc


_NC_CACHE = {}


def _get_nc(S):
    if S not in _NC_CACHE:
        _NC_CACHE[S] = build_nc(S)
    return _NC_CACHE[S]


def kernel(x, qkv_w, qkv_b, proj_w, proj_b, return_res=False, **run_kwargs):
    import ml_dtypes

    x = np.asarray(x, dtype=np.float32)
    qkv_w = np.asarray(qkv_w, dtype=np.float32)
    qkv_b = np.asarray(qkv_b, dtype=np.float32)
    proj_w = np.asarray(proj_w, dtype=np.float32)
    proj_b = np.asarray(proj_b, dtype=np.float32)
    B, S, _ = x.shape

    nc = _get_nc(S)
    bf = ml_dtypes.bfloat16
    x_bf = x.astype(bf)
    in_maps = []
    for c in range(8):
        b, g = c // 2, c % 2
        qs = slice(384 * g, 384 * g + 384)
        ks = slice(768 + 384 * g, 768 + 384 * g + 384)
        vs = slice(1536 + 384 * g, 1536 + 384 * g + 384)
        in_maps.append(
            {
                "x": np.ascontiguousarray(x_bf[b]).view(np.uint16),
                "wqk": np.ascontiguousarray(
                    np.concatenate([qkv_w[:, qs], qkv_w[:, ks]], axis=1).astype(bf)
                ).view(np.uint16),
                "wv": np.ascontiguousarray(qkv_w[:, vs].astype(bf)).view(np.uint16),
                "bqk": np.ascontiguousarray(
                    np.concatenate([qkv_b[qs], qkv_b[ks]])
                ),
                "wp": np.ascontiguousarray(
                    proj_w[384 * g : 384 * g + 384, :].astype(bf)
                ).view(np.uint16),
                "z8": np.zeros((128, 2048), np.uint8),
            }
        )
    try:
        res = run_bass_kernel_spmd(
            nc, in_maps, core_ids=list(range(8)), **run_kwargs
        )
    except Exception:
        # transient NRT/device errors happen occasionally; retry once
        res = run_bass_kernel_spmd(
            nc, in_maps, core_ids=list(range(8)), **run_kwargs
        )
    # effective bias: the v bias passes through softmax additively
    b_eff = (proj_b.astype(np.float64)
             + qkv_b[1536:].astype(np.float64) @ proj_w.astype(np.float64)
             ).astype(np.float32)
    out = np.empty((B, S, HID), np.float32)
    for b in range(B):
        yt = res.results[2 * b]["yT"] + res.results[2 * b + 1]["yT"]
        out[b] = yt.T + b_eff
    if return_res:
        return out, res
    return out


# revision 38
# speedup vs baseline: 1.2929x; 1.0179x over previous
"""MultiHeadAttention Trainium2 kernel (8 NeuronCores).

Sharding: core c -> (batch b = c//2, head-group g = c%2) of the 12 heads.
Each core computes attention for its 6 heads of one batch element and a
partial projection; the host sums the two head-group partials per batch
element and adds the effective proj bias (proj_b + bv @ proj_w; the v bias
is additive after softmax because attention rows sum to 1).

Per-core dataflow (bf16 datapath, fp8 DoubleRow scores):
  x bf16 [S,768] --PE-transpose--> xT bf16 [128,6,S]
  q/k psum f32 = wqk_bf16^T @ xT; DVE adds bias and converts to fp8 in a
    zero-padded DoubleRow layout q8/k8 [64|64 part, pair, 2, S]
  v bf16 seq-major vsl [sk, skpair, j, head, 65] (+ ones col for denom)
  scores[sk,sq] = DoubleRow fp8 matmul (contraction 64 + 64 zeros)
  pt = exp(scores/8): split ACT (exact) / Pool / DVE (Schraudolph bits)
  av[sq,2,65] += pt-chunk^T @ v    (bf16, psum accum; col 64 = denom)
  ao_n = av / denom (broadcast divide, DVE), PE-transpose to aoT [384,S]
  yT[768,S] = wp_bf16^T @ aoT      (partial projection, host sums pairs)
"""
import sys

sys.path.insert(0, "/opt/trn_rl_repo")

import numpy as np

import concourse.bass as bass
import concourse.mybir as mybir
import concourse.tile as tile
from concourse import bacc
from concourse.bass_utils import run_bass_kernel_spmd
from concourse.masks import make_identity

F32 = mybir.dt.float32
BF16 = mybir.dt.bfloat16
U16 = mybir.dt.uint16
FP8 = mybir.dt.float8e4
EXP = mybir.ActivationFunctionType.Exp
COPY_FN = mybir.ActivationFunctionType.Identity
ADD = mybir.AluOpType.add
MULT = mybir.AluOpType.mult
DIV = mybir.AluOpType.divide
DR = mybir.MatmulPerfMode.DoubleRow

HID = 768
D = 64  # head dim
LHEADS = 6  # heads per core
PAIRS = 3

LOG2E = 1.4426950408889634
# Schraudolph-in-bf16-bits: n = x*0.125*128*log2e + (16256 + c); floor via
# the executor's f32->u16 cast. c=-6.85 calibrated for min rms vs exp().
SCHR_MUL = 0.125 * 128.0 * LOG2E
SCHR_ADD = 16256.0 - 6.85

# exp engine split: per block of 16 exp units (1024 rows each).
# GPSIMD (Pool) cannot access PSUM on TRN2, so only ACT and DVE apply.
EXP_PATTERN = (
    "act", "dve", "act", "act", "dve", "act", "dve", "act",
    "act", "dve", "act", "dve", "act", "act", "dve", "act",
)


def build_nc(S: int, taps: bool = False):
    nc = bacc.Bacc("TRN2", target_bir_lowering=False, debug=False)
    NSEQ = S // 128  # seq chunks of 128
    NBLK = S // 512  # seq blocks of 512
    NPAIR = NSEQ // 2  # sk chunk pairs
    XG = 4  # x DMA chunk groups
    NXG = NSEQ // XG

    x = nc.dram_tensor("x", [S, HID], BF16, kind="ExternalInput")
    wqk = nc.dram_tensor("wqk", [HID, 768], BF16, kind="ExternalInput")
    wv = nc.dram_tensor("wv", [HID, 384], BF16, kind="ExternalInput")
    bqk = nc.dram_tensor("bqk", [768], F32, kind="ExternalInput")
    wp = nc.dram_tensor("wp", [384, HID], BF16, kind="ExternalInput")
    z8 = nc.dram_tensor("z8", [128, 2048], FP8, kind="ExternalInput")
    yT = nc.dram_tensor("yT", [HID, S], F32, kind="ExternalOutput")
    if taps:
        t_xT = nc.dram_tensor("t_xT", [128, 6 * S], BF16, kind="ExternalOutput")
        t_q8 = nc.dram_tensor("t_q8", [128, 2 * S], FP8, kind="ExternalOutput")
        t_k8 = nc.dram_tensor("t_k8", [128, 2 * S], FP8, kind="ExternalOutput")
        t_v = nc.dram_tensor("t_v", [128, NSEQ // 2 * 2 * 6 * (D + 1)], BF16, kind="ExternalOutput")
        t_pt = nc.dram_tensor("t_pt", [128, 16 * 2 * 512], BF16, kind="ExternalOutput")
        t_ao = nc.dram_tensor("t_ao", [128, PAIRS * S], BF16, kind="ExternalOutput")
        t_aon = nc.dram_tensor("t_aon", [128, 2 * D], BF16, kind="ExternalOutput")
        t_qp = nc.dram_tensor("t_qp", [128, 2, 512], F32, kind="ExternalOutput")

    with tile.TileContext(nc) as tc:
        with (
            tc.tile_pool(name="const", bufs=1) as cp,
            tc.tile_pool(name="wts", bufs=1) as wpool,
            tc.tile_pool(name="qk8", bufs=1) as qk8p,
            tc.tile_pool(name="ao", bufs=1) as aop,
            tc.tile_pool(name="ps", bufs=1, space="PSUM") as ps,
        ):
            identf = cp.tile([128, 128], F32, tag="identf")
            make_identity(nc, identf[:])
            ident = cp.tile([128, 128], BF16, tag="ident")
            nc.vector.tensor_copy(ident[:], identf[:])
            # q/k bias, feature-major [128, 6] (chunk c: c<3 q, c>=3 k)
            bqk_sb = cp.tile([128, 6], F32, tag="bqk")
            nc.sync.dma_start(bqk_sb[:], bqk[:].rearrange("(c p) -> p c", p=128))
            # load the exp ACT table off the critical path
            warm = cp.tile([1, 16], F32, tag="warm")
            nc.vector.memset(warm[:], 1.0)
            nc.scalar.activation(warm[:], warm[:], EXP, bias=0.0, scale=0.0)

            # fp8 q/k, zero-padded DoubleRow layout, one tile per pair:
            # [128, 2, S]; partition half = head-of-pair, dim1 = j (j=1
            # stays zero) so [64*hi:64*hi+64, :, a:b] is a DR operand.
            q8_tiles = {}
            k8_tiles = {}

            def get_qk8(p):
                if p not in q8_tiles:
                    q8_p = qk8p.tile([128, 2, S], FP8, tag="q8", bufs=3,
                                     name=f"q8_{p}")
                    k8_p = qk8p.tile([128, 2, S], FP8, tag="k8", bufs=3,
                                     name=f"k8_{p}")
                    nc.sync.dma_start(q8_p[:, 1, :], z8[:])
                    nc.sync.dma_start(k8_p[:, 1, :], z8[:])
                    q8_tiles[p] = q8_p
                    k8_tiles[p] = k8_p
                return q8_tiles[p], k8_tiles[p]

            aoT = aop.tile([128, PAIRS, S], BF16, tag="aoT")
            yT_ap = yT[:].rearrange("(c p) s -> p c s", p=128)

            with tc.tile_pool(name="xT", bufs=1) as xtp, \
                 tc.tile_pool(name="vv", bufs=1) as vvp:
                xT = xtp.tile([128, 6, S], BF16, tag="xT")
                # v seq-major [sk, skpair, j, head, 65]; col 64 = ones
                vsl = vvp.tile([128, NPAIR, 2, LHEADS, D + 1], BF16, tag="v")
                nc.vector.memset(vsl[:, :, :, :, D : D + 1], 1.0)

                # --- x DMA, weights, transposes ---
                with tc.tile_pool(name="xin", bufs=1) as xin:
                    x_ap = x[:].rearrange("(n p) d -> p n d", p=128)
                    x_ts = []
                    for g in range(XG):
                        x_t = xin.tile([128, NXG, HID], BF16, tag=f"x{g}", name=f"x_t{g}")
                        x_ts.append(x_t)
                    nc.sync.dma_start(x_ts[0][:], x_ap[:, 0:NXG, :])
                    wqk_r = wpool.tile([128, 6, 768], BF16, tag="wqkr")
                    wqk_ap = wqk[:].rearrange("(c p) f -> p c f", p=128)
                    for kc in range(6):
                        nc.sync.dma_start(
                            wqk_r[:, kc : kc + 1, :], wqk_ap[:, kc : kc + 1, :]
                        )
                    for g in range(1, XG):
                        nc.sync.dma_start(
                            x_ts[g][:], x_ap[:, g * NXG : (g + 1) * NXG, :]
                        )
                    wv_r = wpool.tile([128, 6, 384], BF16, tag="wvr")
                    nc.sync.dma_start(
                        wv_r[:], wv[:].rearrange("(c p) f -> p c f", p=128)
                    )
                    wp_r = wpool.tile([128, 3, HID], BF16, tag="wpr")
                    nc.sync.dma_start(
                        wp_r[:], wp[:].rearrange("(c p) f -> p c f", p=128)
                    )

                    # transposes: per x-group, per hid-chunk j, 4 seq chunks
                    # into one psum tile, then one contiguous copy
                    for g in range(XG):
                        for j in range(6):
                            tp = ps.tile([128, NXG, 128], BF16, tag="sc", bufs=3)
                            for i in range(NXG):
                                nc.tensor.transpose(
                                    tp[:, i, :],
                                    x_ts[g][:, i, j * 128 : (j + 1) * 128],
                                    ident[:],
                                )
                            dst = xT[:, j, g * NXG * 128 : (g + 1) * NXG * 128]
                            if (g * 6 + j) % 2 == 0:
                                nc.scalar.copy(dst, tp[:])
                            else:
                                nc.vector.tensor_copy(dst, tp[:])

                smp_holder = [None]

                def emit_qk(p, n):
                    """q+k psum for pair p, block n -> fp8 padded layout."""
                    qp = ps.tile([128, 2, 512], F32, tag="sc", bufs=3)
                    for qk_i, wcol in ((0, p), (1, 3 + p)):
                        for kc in range(6):
                            nc.tensor.matmul(
                                qp[:, qk_i, :],
                                wqk_r[:, kc, wcol * 128 : (wcol + 1) * 128],
                                xT[:, kc, n * 512 : (n + 1) * 512],
                                start=(kc == 0),
                                stop=(kc == 5),
                                skip_group_check=True,
                            )
                    sl = slice(n * 512, (n + 1) * 512)
                    if taps and p == 0 and n == 0:
                        qp_sb = cp.tile([128, 2, 512], F32, tag="qptap", name="qp_sb")
                        nc.vector.tensor_copy(qp_sb[:], qp[:])
                        nc.sync.dma_start(t_qp[:], qp_sb[:])
                    q8_p, k8_p = get_qk8(p)
                    for dst8, qk_i, wcol in ((q8_p, 0, p), (k8_p, 1, 3 + p)):
                        for hi in range(2):
                            if (qk_i + hi) % 2 == 0:
                                nc.vector.tensor_scalar(
                                    dst8[64 * hi : 64 * hi + 64, 0, sl],
                                    qp[64 * hi : 64 * hi + 64, qk_i, :],
                                    bqk_sb[64 * hi : 64 * hi + 64, wcol : wcol + 1],
                                    None,
                                    ADD,
                                )
                            else:
                                nc.scalar.activation(
                                    dst8[64 * hi : 64 * hi + 64, 0, sl],
                                    qp[64 * hi : 64 * hi + 64, qk_i, :],
                                    COPY_FN,
                                    bias=bqk_sb[64 * hi : 64 * hi + 64, wcol : wcol + 1],
                                    scale=1.0,
                                )

                def emit_v(i):
                    """v for seq chunk i, all 6 heads, seq-major, bias-free."""
                    vp = ps.tile([128, 2, 512], F32, tag="sc", bufs=3)
                    for kc in range(6):
                        nc.tensor.matmul(
                            vp[:, 0, 0:384],
                            xT[:, kc, i * 128 : (i + 1) * 128],
                            wv_r[:, kc, :],
                            start=(kc == 0),
                            stop=(kc == 5),
                            skip_group_check=True,
                        )
                    nc.vector.tensor_copy(
                        vsl[:, i // 2, i % 2, :, 0:D],
                        vp[:, 0, 0:384].rearrange("p (h d) -> p h d", h=6),
                    )

                with (
                    tc.tile_pool(name="pt", bufs=1) as ptp,
                    tc.tile_pool(name="sm", bufs=1) as smp,
                ):
                    smp_holder[0] = smp
                    exp_idx = [0]

                    def emit_score_unit(p, n, pt, hi, i):
                        q8_p, k8_p = get_qk8(p)
                        sc = ps.tile([128, 2, 512], F32, tag="sc", bufs=3,
                                     name="sc")
                        for j in range(2):
                            sk = 2 * i + j
                            nc.tensor.matmul(
                                sc[:, j, :],
                                k8_p[64 * hi : 64 * hi + 64, :,
                                     sk * 128 : (sk + 1) * 128],
                                q8_p[64 * hi : 64 * hi + 64, :,
                                     n * 512 : (n + 1) * 512],
                                start=True,
                                stop=True,
                                perf_mode=DR,
                            )
                        dst = pt[:, 8 * hi + i, :, :]
                        eng = EXP_PATTERN[exp_idx[0] % len(EXP_PATTERN)]
                        exp_idx[0] += 1
                        if eng == "act":
                            nc.scalar.activation(
                                dst, sc[:], EXP, bias=0.0, scale=0.125
                            )
                        else:
                            nc.vector.tensor_scalar(
                                dst.bitcast(U16),
                                sc[:],
                                SCHR_MUL, SCHR_ADD, MULT, ADD,
                            )

                    def emit_scores(p, n, pt, hook=None):
                        u = 0
                        for hi in range(2):
                            for i in range(NPAIR):
                                emit_score_unit(p, n, pt, hi, i)
                                if hook is not None and u % 4 == 3:
                                    hook(u // 4)
                                u += 1

                    ao_ns = {}

                    def emit_av_chunk(p, n, pt, c):
                            av = ps.tile([128, 2, D + 1], F32, tag="av", bufs=2)
                            first = True
                            for hi in range(2):
                                for i in range(NPAIR):
                                    for j in range(2):
                                        nc.tensor.matmul(
                                            av[:, hi, :],
                                            pt[:, 8 * hi + i, j,
                                               c * 128 : (c + 1) * 128],
                                            vsl[:, i, j, 2 * p + hi, :],
                                            start=first,
                                            stop=(hi == 1 and i == NPAIR - 1
                                                  and j == 1),
                                            skip_group_check=True,
                                        )
                                        first = False
                            rec = smp.tile([128, 2], F32, tag="rec", bufs=4)
                            nc.vector.reciprocal(rec[:], av[:, :, D])
                            ao_n = smp.tile([128, 2, D], BF16, tag="aon", bufs=32,
                                            name=f"ao_n_{p}_{n}_{c}")
                            nc.vector.tensor_tensor(
                                ao_n[:],
                                av[:, :, 0:D],
                                rec[:].unsqueeze(2).broadcast_to([128, 2, D]),
                                MULT,
                            )
                            ao_ns[(p, n, c)] = ao_n

                    def emit_av(p, n, pt):
                        for c in range(4):
                            emit_av_chunk(p, n, pt, c)

                    def emit_aoT_block(p, n):
                        # transpose one block's normalized outputs into aoT
                        tp2 = ps.tile([128, 4, 128], BF16, tag="av",
                                      bufs=2, name=f"tp2_{p}_{n}")
                        for c in range(4):
                            nc.tensor.transpose(
                                tp2[:, c, :],
                                ao_ns.pop((p, n, c))[:].rearrange(
                                    "p h d -> p (h d)"),
                                ident[:],
                            )
                        nc.vector.tensor_copy(
                            aoT[:, p, n * 512 : (n + 1) * 512],
                            tp2[:].rearrange("p a b -> p (a b)"),
                        )

                    def emit_proj_block(n):
                        for m in range(6):
                            pp = ps.tile([128, 2, 512], F32, tag="sc", bufs=3,
                                         name="pp")
                            for kc in range(3):
                                nc.tensor.matmul(
                                    pp[:, 0, :],
                                    wp_r[:, kc, m * 128 : (m + 1) * 128],
                                    aoT[:, kc, n * 512 : (n + 1) * 512],
                                    start=(kc == 0),
                                    stop=(kc == 2),
                                    skip_group_check=True,
                                )
                            yt_t = smp.tile([128, 512], F32, tag="yT", bufs=6, name="yt_t")
                            if m % 2 == 0:
                                nc.scalar.copy(yt_t[:], pp[:, 0, :])
                            else:
                                nc.vector.tensor_copy(yt_t[:], pp[:, 0, :])
                            nc.sync.dma_start(
                                yT_ap[:, m, n * 512 : (n + 1) * 512], yt_t[:]
                            )

                    if taps:
                        def tap(dram, ap):
                            nc.sync.dma_start(dram[:], ap)
                    pt_tiles = {}
                    prev = None
                    aoT_pending = []
                    for p in range(PAIRS):
                        for n in range(NBLK):
                            pt = ptp.tile([128, 16, 2, 512], BF16, tag="pt",
                                          bufs=2, name=f"pt_{p}_{n}")
                            pt_tiles[(p, n)] = pt
                            # interleave prev block's AV chunks into this
                            # block's score stream (safe once v is complete)
                            hook = None
                            if prev is not None and not (p == 0 and n <= 1):
                                pp_, pn_, ppt_ = prev
                                hook = lambda c: emit_av_chunk(pp_, pn_, ppt_, c)
                            if p == 0 and n == 0:
                                # k for the whole pair arrives per qk block;
                                # emit score units as their sk range lands
                                for kb in range(NBLK):
                                    emit_qk(0, kb)
                                    for i in (2 * kb, 2 * kb + 1):
                                        for hi in range(2):
                                            emit_score_unit(0, 0, pt, hi, i)
                            else:
                                emit_scores(p, n, pt, hook=hook)
                            if taps and p == 0 and n == 0:
                                tap(t_xT, xT[:].rearrange("p a b -> p (a b)"))
                                tap(t_q8, q8_tiles[0][:].rearrange("p b c -> p (b c)"))
                                tap(t_k8, k8_tiles[0][:].rearrange("p b c -> p (b c)"))
                                tap(t_v, vsl[:].rearrange("p a b c d -> p (a b c d)"))
                                tap(t_pt, pt[:].rearrange("p a b c -> p (a b c)"))
                            if p == 0 and n < 2:
                                for i in range(8 * n, 8 * n + 8):
                                    emit_v(i)
                            if p < PAIRS - 1:
                                emit_qk(p + 1, n)
                            if prev is not None and hook is None:
                                emit_av(*prev)
                            if prev is not None:
                                aoT_pending.append(prev[:2])
                            # emit one pending aoT block, one block lagged
                            # so its norms have drained
                            if len(aoT_pending) >= 2:
                                key = aoT_pending.pop(0)
                                emit_aoT_block(*key)
                                if key[0] == PAIRS - 1:
                                    emit_proj_block(key[1])
                            prev = (p, n, pt)
                    emit_av(*prev)
                    aoT_pending.append(prev[:2])
                    # drain remaining aoT blocks; start projection per sq
                    # block as soon as all three pairs' aoT rows exist
                    for key in aoT_pending:
                        emit_aoT_block(*key)
                        if key[0] == PAIRS - 1:
                            emit_proj_block(key[1])
                    if taps:
                        tap(t_ao, aoT[:].rearrange("p a b -> p (a b)"))

    nc.finalize()
    return nc


_NC_CACHE = {}


def _get_nc(S):
    if S not in _NC_CACHE:
        _NC_CACHE[S] = build_nc(S)
    return _NC_CACHE[S]


def kernel(x, qkv_w, qkv_b, proj_w, proj_b, return_res=False, **run_kwargs):
    import ml_dtypes

    x = np.asarray(x, dtype=np.float32)
    qkv_w = np.asarray(qkv_w, dtype=np.float32)
    qkv_b = np.asarray(qkv_b, dtype=np.float32)
    proj_w = np.asarray(proj_w, dtype=np.float32)
    proj_b = np.asarray(proj_b, dtype=np.float32)
    B, S, _ = x.shape

    nc = _get_nc(S)
    bf = ml_dtypes.bfloat16
    x_bf = x.astype(bf)
    in_maps = []
    for c in range(8):
        b, g = c // 2, c % 2
        qs = slice(384 * g, 384 * g + 384)
        ks = slice(768 + 384 * g, 768 + 384 * g + 384)
        vs = slice(1536 + 384 * g, 1536 + 384 * g + 384)
        in_maps.append(
            {
                "x": np.ascontiguousarray(x_bf[b]).view(np.uint16),
                "wqk": np.ascontiguousarray(
                    np.concatenate([qkv_w[:, qs], qkv_w[:, ks]], axis=1).astype(bf)
                ).view(np.uint16),
                "wv": np.ascontiguousarray(qkv_w[:, vs].astype(bf)).view(np.uint16),
                "bqk": np.ascontiguousarray(
                    np.concatenate([qkv_b[qs], qkv_b[ks]])
                ),
                "wp": np.ascontiguousarray(
                    proj_w[384 * g : 384 * g + 384, :].astype(bf)
                ).view(np.uint16),
                "z8": np.zeros((128, 2048), np.uint8),
            }
        )
    try:
        res = run_bass_kernel_spmd(
            nc, in_maps, core_ids=list(range(8)), **run_kwargs
        )
    except Exception:
        # transient NRT/device errors happen occasionally; retry once
        res = run_bass_kernel_spmd(
            nc, in_maps, core_ids=list(range(8)), **run_kwargs
        )
    # effective bias: the v bias passes through softmax additively
    b_eff = (proj_b.astype(np.float64)
             + qkv_b[1536:].astype(np.float64) @ proj_w.astype(np.float64)
             ).astype(np.float32)
    out = np.empty((B, S, HID), np.float32)
    for b in range(B):
        yt = res.results[2 * b]["yT"] + res.results[2 * b + 1]["yT"]
        out[b] = yt.T + b_eff
    if return_res:
        return out, res
    return out
